# revision 1
# baseline (speedup 1.0000x reference)
"""Trainium2 Bass kernel for nn_Downsample_Spa: sigma-conv + gaussian unfold downsample.

Math (per batch image, one NeuronCore each; batch of 8 -> 8 cores):
  xp = reflect_pad(x)                                  # [64,130,130]
  sigma[o,p] = clamp(BN(conv3x3(xp))[o,p], 1e-4)       # at stride-2 positions p only
  graw[o,p]  = exp(-0.5*d2[o]/sigma^2 - ln64) / sigma  # /64 guards fp16 range; cancels in the ratio
  out[c,p]   = sum_o graw[o,p]*xp[c,p+off(o)] / sum_o graw[o,p]

Design:
 - partitions = (row-half hh, channel c) = 128; host pre-pads (reflect) and stores x
   in fp16 with columns parity-split into 3 planes (w=2j / w=2j+1 / w=2j+2) so every
   tap is a step-1 AP (enables PE full rate + DVE 2x modes); ~0.5 ulp fp16 noise.
 - conv: 9 accumulating fp16 matmuls per 512-position block, block-diagonal weights
   (M=18 computes both row halves per N-stream). sigma in fp32 PSUM.
 - g pipeline (fp32): clamp via tensor_scalar(add bias, max eps), custom-DVE fast
   reciprocal, ACT Square + Exp (one table set), -> gb fp16.
 - unfold: per tap a one-hot fp16 matmul broadcasts gb across the 64 channel
   partitions; taps are packed 3/2-wide, copied to SBUF fp16 (ACT), multiplied with
   x (DVE 2x), pair-tree summed (fp16), normalized by 1/sum (10th all-ones matmul +
   fast reciprocal); the center tap stays fp32 end-to-end.
"""

import os
import sys

import numpy as np

if "/opt/trn_rl_repo" not in sys.path:
    sys.path.insert(0, "/opt/trn_rl_repo")

K = 3
BN_EPS = 1e-5
SIGMA_MIN = 1e-4
GSCALE_LN = float(np.log(64.0))   # graw scaled by 1/64 (folded into exp bias)
N, C, H, W = 8, 64, 128, 128
HO = WO = 64
HH = 2
RS = 65                  # padded-row slots per partition-half
HOC = 32
NBLK = 4
BR = HOC // NBLK         # 8 output rows per block
NPOS = BR * WO           # 512
NP2 = 2 * NPOS
PL = 3                   # x col-parity planes: w=2j / w=2j+1 / w=2j+2
JW = 66                  # j slots per plane (65 used max, 66 for alignment)
CR = 17                  # rows per DMA chunk tile (16 + 1 overlap)

# f32 consts tensor columns
_D2 = 0                  # -0.5*d2[o] per (hh,o)
_BC = 1                  # bn_bias - sigma_min
_LB = 2                  # exp bias: constant -ln(64) per partition
_NCC = 3

_STATE = {}


def _build_consts(conv_w, bn_gamma, bn_beta, bn_mean, bn_var):
    s = (bn_gamma / np.sqrt(bn_var + BN_EPS)).astype(np.float32)
    wf = conv_w.astype(np.float32) * s[:, None, None, None]           # [9,64,3,3]
    bias = (bn_beta - bn_mean * s).astype(np.float32)

    cst = np.zeros((18, _NCC), np.float32)
    d2 = np.array([(kk // 3 - 1) ** 2 + (kk % 3 - 1) ** 2 for kk in range(9)], np.float32)
    for hh in range(HH):
        cst[hh * 9:hh * 9 + 9, _D2] = -0.5 * d2
        cst[hh * 9:hh * 9 + 9, _BC] = bias - SIGMA_MIN
        cst[hh * 9:hh * 9 + 9, _LB] = -GSCALE_LN

    # conv weights, block-diagonal per tap: win[k=hh*64+c, tap*18 + hh*9+o]
    win = np.zeros((128, 9 * 18), np.float16)
    for tap in range(9):
        i, j = tap // 3, tap % 3
        for hh in range(HH):
            win[hh * 64:hh * 64 + 64, tap * 18 + hh * 9:tap * 18 + hh * 9 + 9] = \
                wf[:, :, i, j].T.astype(np.float16)

    # one-hot / ones broadcast weights: gin[k=hh*9+o, tap*128 + hh*64+c]
    import ml_dtypes
    gin = np.zeros((18, 10 * 128), ml_dtypes.bfloat16)
    for hh in range(HH):
        gin[hh * 9:hh * 9 + 9, 9 * 128 + hh * 64:9 * 128 + hh * 64 + 64] = 1.0
        for tap in range(9):
            gin[hh * 9 + tap, tap * 128 + hh * 64:tap * 128 + hh * 64 + 64] = 1.0
    return cst, win, gin


def _build_bass(for_sim=False):
    import concourse.bass as bass
    import concourse.tile as tile
    from concourse import mybir

    f32 = mybir.dt.float32
    f16 = mybir.dt.float16
    bf16 = mybir.dt.bfloat16
    MULT = mybir.AluOpType.mult
    ADD = mybir.AluOpType.add
    MAX = mybir.AluOpType.max
    AF = mybir.ActivationFunctionType

    if for_sim:
        nc = bass.Bass("TRN2", target_bir_lowering=False, detect_race_conditions=False)
    else:
        from concourse import bacc
        nc = bacc.Bacc()
    xin = nc.dram_tensor("xin", [128, RS, PL, JW], f16, kind="ExternalInput")
    cin = nc.dram_tensor("cin", [18, _NCC], f32, kind="ExternalInput")
    win = nc.dram_tensor("win", [128, 9 * 18], f16, kind="ExternalInput")
    gin = nc.dram_tensor("gin", [18, 10 * 128], bf16, kind="ExternalInput")
    out = nc.dram_tensor("out", [128, HOC, WO], f32, kind="ExternalOutput")

    with tile.TileContext(nc) as tc:
        from contextlib import ExitStack
        with ExitStack() as ctx:
            big = ctx.enter_context(tc.tile_pool(name="big", bufs=1))
            gsb = ctx.enter_context(tc.tile_pool(name="gsb", bufs=3))
            y_p = ctx.enter_context(tc.tile_pool(name="y", bufs=3))
            ps_s = ctx.enter_context(tc.tile_pool(name="ps_s", bufs=2, space="PSUM"))
            ps_g = ctx.enter_context(tc.tile_pool(name="ps_g", bufs=2, space="PSUM"))

            ws = big.tile([128, 9 * 18], f16)
            nc.sync.dma_start(out=ws[:], in_=win[:])
            cs = big.tile([18, _NCC], f32)
            nc.gpsimd.dma_start(out=cs[:], in_=cin[:])
            gs = big.tile([18, 10 * 128], bf16)
            nc.gpsimd.dma_start(out=gs[:], in_=gin[:])

            xsk = []
            for blk in range(NBLK):
                xs = big.tile([128, CR, PL, JW], f16, tag=f"xs{blk}")
                nc.sync.dma_start(out=xs[:], in_=xin[:, 16 * blk:16 * blk + CR, :, :])
                xsk.append(xs)

            def xtap(tap, blk):
                # [128, 8, 64] fp16 step-1 view for tap (i,b) in block blk
                i, b = tap // 3, tap % 3
                return xsk[blk][:, i:i + 2 * BR - 1:2, b, 0:WO]

            def xtaps(i, pls, blk):
                # [128, nplanes, 8, 64] multi-tap view (planes outer, rows, cols)
                return xsk[blk][:, i:i + 2 * BR - 1:2, pls, 0:WO].transpose([0, 2, 1, 3])

            # ---- PE warm-up: dummy matmuls on the (early-arriving) weights tile
            # during the input-DMA wait, so HAM un-throttles before the conv ----
            wu = ps_s.tile([18, NPOS], f32, tag="sig")
            for _ in range(12):
                nc.tensor.matmul(wu[:, 0:162], ws[:, 0:18], ws[:, 0:162],
                                 start=True, stop=True)

            # ---- conv: per-block sigma [18,512] in PSUM, clamped to a [18,1024] SBUF half ----
            sigh = []
            for h in range(2):
                sc = gsb.tile([18, NP2], f32, tag=f"sc{h}")
                for sub in range(2):
                    blk = 2 * h + sub
                    sig = ps_s.tile([18, NPOS], f32, tag="sig")
                    for tap in range(9):
                        nc.tensor.matmul(
                            sig[:],
                            ws[:, tap * 18:(tap + 1) * 18],
                            xtap(tap, blk),
                            start=(tap == 0), stop=(tap == 8),
                        )
                    nc.vector.tensor_scalar(out=sc[:, sub * NPOS:(sub + 1) * NPOS],
                                            in0=sig[:],
                                            scalar1=cs[:, _BC:_BC + 1],
                                            scalar2=float(SIGMA_MIN),
                                            op0=ADD, op1=MAX)
                sigh.append(sc)

            def g_emit(sc):
                inv = gsb.tile([18, NP2], f32, tag="inv")
                nc.vector.reciprocal_approx_fast(out=inv[:], in_=sc[:])
                qt = gsb.tile([18, NP2], f32, tag="qt")
                nc.scalar.activation(out=qt[:], in_=inv[:], func=AF.Square)
                et = gsb.tile([18, NP2], f32, tag="et")
                nc.scalar.activation(out=et[:], in_=qt[:], func=AF.Exp,
                                     scale=cs[:, _D2:_D2 + 1],
                                     bias=cs[:, _LB:_LB + 1])
                gb = gsb.tile([18, NP2], bf16, tag="gb")
                nc.vector.tensor_tensor(out=gb[:], in0=et[:], in1=inv[:], op=MULT)
                return gb

            def unfold_emit(blk, gbs):
                # normalizer (10th tap): Srep[(hh,c),p] = sum_o gb
                Srep = ps_s.tile([128, NPOS], f32, tag="sig")
                nc.tensor.matmul(Srep[:], gs[:, 9 * 128:10 * 128],
                                 gbs, start=True, stop=True)
                rr = gsb.tile([128, NPOS], f32, tag="rr")
                nc.vector.reciprocal_approx_fast(out=rr[:], in_=Srep[:])

                # tap groups: row0 triple (0,1,2) / row1 (3,5)+center 4 / row2 triple (6,7,8)
                yt = y_p.tile([128, 8, BR, WO], f16, tag="yt")
                y4 = y_p.tile([128, BR, WO], f32, tag="y4")

                def tap_group(taps, slot0, i, pls):
                    g3 = ps_g.tile([128, 3, NPOS], f32, tag="grep")
                    for k, tap in enumerate(taps):
                        nc.tensor.matmul(g3[:, k, :], gs[:, tap * 128:(tap + 1) * 128],
                                         gbs, start=True, stop=True)
                    gc = y_p.tile([128, len(taps), BR, WO], f16, tag=f"gc{slot0}")
                    nc.scalar.activation(out=gc[:], in_=g3[:, 0:len(taps), :], func=AF.Copy)
                    nc.vector.tensor_tensor(
                        out=yt[:, slot0:slot0 + len(taps)],
                        in0=xtaps(i, pls, blk), in1=gc[:], op=MULT)

                tap_group((0, 1, 2), 0, 0, slice(0, 3))
                tap_group((6, 7, 8), 5, 2, slice(0, 3))
                # row1: pair (3,5) at planes 0,2 + fp32 center (4)
                g2 = ps_g.tile([128, 3, NPOS], f32, tag="grep")
                nc.tensor.matmul(g2[:, 0, :], gs[:, 3 * 128:4 * 128], gbs, start=True, stop=True)
                nc.tensor.matmul(g2[:, 2, :], gs[:, 5 * 128:6 * 128], gbs, start=True, stop=True)
                nc.tensor.matmul(g2[:, 1, :], gs[:, 4 * 128:5 * 128], gbs, start=True, stop=True)
                gc2 = y_p.tile([128, 2, BR, WO], f16, tag="gc2p")
                nc.scalar.activation(out=gc2[:], in_=g2[:, 0:3:2, :], func=AF.Copy)
                nc.vector.tensor_tensor(out=yt[:, 3:5], in0=xtaps(1, slice(0, 3, 2), blk),
                                        in1=gc2[:], op=MULT)
                nc.vector.tensor_tensor(out=y4[:], in0=xtap(4, blk), in1=g2[:, 1, :], op=MULT)

                # pair tree (fp16) + center + normalize
                t4 = y_p.tile([128, 4, BR, WO], f16, tag="t4")
                nc.vector.tensor_tensor(out=t4[:], in0=yt[:, 0:8:2], in1=yt[:, 1:8:2], op=ADD)
                late = blk == NBLK - 1
                t2 = y_p.tile([128, 2, BR, WO], f16, tag="t2")
                nc.vector.tensor_tensor(out=t2[:], in0=t4[:, 0:4:2], in1=t4[:, 1:4:2], op=ADD)
                t1 = y_p.tile([128, BR, WO], f16, tag="t1")
                (nc.vector if late else nc.gpsimd).tensor_tensor(out=t1[:], in0=t2[:, 0], in1=t2[:, 1], op=ADD)
                t0 = y_p.tile([128, BR, WO], f32, tag="t0")
                (nc.vector if late else nc.gpsimd).tensor_tensor(out=t0[:], in0=t1[:], in1=y4[:], op=ADD)
                acc = y_p.tile([128, BR, WO], f32, tag="acc")
                (nc.vector if late else nc.gpsimd).tensor_tensor(out=acc[:], in0=t0[:], in1=rr[:], op=MULT)
                nc.sync.dma_start(out=out[:, BR * blk:BR * (blk + 1), :], in_=acc[:])

            gb0 = g_emit(sigh[0])
            unfold_emit(0, gb0[:, 0:NPOS])
            gb1 = g_emit(sigh[1])
            unfold_emit(1, gb0[:, NPOS:NP2])
            unfold_emit(2, gb1[:, 0:NPOS])
            unfold_emit(3, gb1[:, NPOS:NP2])

    if not for_sim and not nc.is_finalized():
        nc.finalize()
    return nc


def _prep_inputs(x, conv_w, bn_gamma, bn_beta, bn_mean, bn_var):
    cst, win, gin = _build_consts(conv_w, bn_gamma, bn_beta, bn_mean, bn_var)
    xp = np.pad(np.asarray(x, np.float32), ((0, 0), (0, 0), (1, 1), (1, 1)),
                mode="reflect").astype(np.float16)                    # [8,64,130,130]
    in_maps = []
    for n in range(N):
        xc = np.concatenate([xp[n, :, 0:RS, :], xp[n, :, 64:64 + RS, :]], axis=0)
        xpl = np.zeros((128, RS, PL, JW), np.float16)
        xpl[:, :, 0, 0:65] = xc[:, :, 0:130:2]
        xpl[:, :, 1, 0:65] = xc[:, :, 1:130:2]
        xpl[:, :, 2, 0:64] = xc[:, :, 2:130:2]
        in_maps.append({"xin": xpl, "cin": cst, "win": win, "gin": gin})
    return in_maps


def _gather(results):
    out = np.empty((N, C, HO, WO), np.float32)
    for n in range(N):
        d = results[n]["out"]
        out[n, :, 0:HOC, :] = d[0:64]
        out[n, :, HOC:, :] = d[64:128]
    return out


def _enable_axon_trace():
    """Register the NTFF profile hook that this image's antenv lacks."""
    if _STATE.get("trace_hooked"):
        return
    import types
    import antenv
    from concourse import bass_utils
    mod = types.ModuleType("antenv.axon_hooks")
    mod._hook = None
    mod.set_axon_ntff_profile_hook = lambda h: setattr(mod, "_hook", h)
    mod.get_axon_ntff_profile_hook = lambda: mod._hook
    sys.modules["antenv.axon_hooks"] = mod
    antenv.axon_hooks = mod
    from trn_agent_boot.trn_boot import _ntff_profile_via_ctypes
    mod._hook = _ntff_profile_via_ctypes("/opt/axon/libaxon_pjrt.so")
    bass_utils.upload_artifacts = lambda tmpdir: tmpdir
    _STATE["trace_hooked"] = True


def run(x, conv_w, bn_gamma, bn_beta, bn_mean, bn_var, trace=False):
    from concourse.bass_utils import run_bass_kernel_spmd
    if trace:
        _enable_axon_trace()
    if "nc" not in _STATE:
        _STATE["nc"] = _build_bass()
    in_maps = _prep_inputs(x, conv_w, bn_gamma, bn_beta, bn_mean, bn_var)
    res = run_bass_kernel_spmd(_STATE["nc"], in_maps, list(range(N)), trace=trace)
    _STATE["last"] = res
    return _gather(res.results)


def kernel(x, conv_w, bn_gamma, bn_beta, bn_mean, bn_var):
    return run(x, conv_w, bn_gamma, bn_beta, bn_mean, bn_var,
               trace=bool(int(os.environ.get("KERNEL_TRACE", "0"))))



# revision 2
# speedup vs baseline: 1.0142x; 1.0142x over previous
"""Trainium2 Bass kernel v2 for nn_Downsample_Spa.

Design:
 - conv emits sigma packed [Mg, 512] per group (group A = block 0, group B =
   blocks 1-3; rows = (b2, hh, o)) via zero-padded block-diagonal lhsT slices;
   the g-pipeline runs once per group. Asymmetric groups shorten the serial
   head: block 0's sigma + g-chain complete while conv B still runs.
 - normalizer folded pre-broadcast: S = ones-matmul over the 9 taps, gbn = gb/S
   in fp16; no [128]-wide reciprocal / final multiply per block.
 - g broadcast to 128 partitions: taps 0-5 via 6 PE one-hot matmuls + 2 ACT
   triple-copies (PSUM fp32 -> SBUF fp16); taps 6-8 via stride-0 DRAM broadcast
   DMA from a 2-replica scratch copy of gbn (replicas dodge HBM bank conflicts).
 - unfold: all-fp16 DVE products (2x mode) + pairwise tree, all on Vector —
   GpSimd tensor ops contend with DVE on SBUF (measured 3-5x slowdown), so
   GpSimd only issues DMAs. Scalar queue issues no DMAs (pure ACT compute).
 - fp16 DMA out (host upconverts); 12 PE warmup matmuls on an early tiny weight
   slice cover the p-state ramp (2.4GHz after ~3us of continuous PE busy).
"""

import os
import sys

import numpy as np

if "/opt/trn_rl_repo" not in sys.path:
    sys.path.insert(0, "/opt/trn_rl_repo")

K = 3
BN_EPS = 1e-5
SIGMA_MIN = 1e-4
N, C, H, W = 8, 64, 128, 128
HO = WO = 64
HH = 2
RS = 65                  # padded-row slots per partition-half
HOC = 32
NBLK = 4
BR = HOC // NBLK         # 8 output rows per block
NPOS = BR * WO           # 512
PL = 3                   # x col-parity planes: w=2j / w=2j+1 / w=2j+2
JW = 66                  # j slots per plane
CR = 17                  # rows per DMA chunk tile (16 + 1 overlap)
GRP = [[0, 1], [2, 3]]   # conv groups (two pipelined g-chains)
MG = [32 * (len(g) - 1) + 18 for g in GRP]   # banded rows (band b2 at 32*b2)
MX = max(MG)             # 114
NWU = 12                 # PE warmup matmuls
REP = 2                  # DRAM replicas of gbn for the tap-6..8 broadcast DMA

_STATE = {}


def _build_consts(conv_w, bn_gamma, bn_beta, bn_mean, bn_var):
    s = (bn_gamma / np.sqrt(bn_var + BN_EPS)).astype(np.float32)
    wf = conv_w.astype(np.float32) * s[:, None, None, None]           # [9,64,3,3]
    bias = (bn_beta - bn_mean * s).astype(np.float32)
    d2 = np.array([(kk // 3 - 1) ** 2 + (kk % 3 - 1) ** 2 for kk in range(9)],
                  np.float32)

    # cs: per-group banded rows concatenated (pads zero)
    # col0 = -0.5*d2[o] (exp scale), col1 = bn_bias - eps
    cs = np.zeros((sum(MG), 2), np.float32)
    off = 0
    for gi, blocks in enumerate(GRP):
        for b2 in range(len(blocks)):
            for hh in range(HH):
                r0 = off + 32 * b2 + hh * 9
                cs[r0:r0 + 9, 0] = -0.5 * d2
                cs[r0:r0 + 9, 1] = bias - SIGMA_MIN
        off += MG[gi]

    # shared conv lhsT [128, tap, 18]: rows (hh, c) -> cols (hh, o)
    wt = np.zeros((128, 9, 18), np.float16)
    for tap in range(9):
        i, j = tap // 3, tap % 3
        for hh in range(HH):
            wt[hh * 64:hh * 64 + 64, tap, hh * 9:hh * 9 + 9] = \
                wf[:, :, i, j].T.astype(np.float16)
    wt = wt.reshape(128, 9 * 18)

    oh1s, ones = [], []
    for gi, blocks in enumerate(GRP):
        nb = len(blocks)
        mg = 32 * (nb - 1) + 18          # banded rows: band b2 at 32*b2
        # broadcast one-hots [mg, (tap0..5, b2), 128]
        oh = np.zeros((mg, 6, nb, 128), np.float16)
        for tap in range(6):
            for b2 in range(nb):
                for hh in range(HH):
                    oh[32 * b2 + hh * 9 + tap, tap, b2, hh * 64:hh * 64 + 64] = 1.0
        oh1s.append(oh.reshape(mg, 6 * nb * 128))

        # S ones [mg, mg]: block-diag 9-tap groups on real rows, identity on pads
        on = np.eye(mg, dtype=np.float16)
        for b2 in range(nb):
            for hh in range(HH):
                r0 = 32 * b2 + hh * 9
                on[r0:r0 + 9, r0:r0 + 9] = 1.0
        ones.append(on)
    return cs, wt, oh1s, ones


def _build_bass(for_sim=False):
    import concourse.bass as bass
    import concourse.tile as tile
    from concourse import mybir

    f32 = mybir.dt.float32
    f16 = mybir.dt.float16
    MULT = mybir.AluOpType.mult
    ADD = mybir.AluOpType.add
    MAX = mybir.AluOpType.max
    AF = mybir.ActivationFunctionType

    if for_sim:
        nc = bass.Bass("TRN2", target_bir_lowering=False, detect_race_conditions=False)
    else:
        from concourse import bacc
        nc = bacc.Bacc()
    xin = nc.dram_tensor("xin", [128, RS, 2, JW], f16, kind="ExternalInput")
    cin = nc.dram_tensor("cin", [sum(MG), 2], f32, kind="ExternalInput")
    win = nc.dram_tensor("win", [128, 9 * 18], f16, kind="ExternalInput")
    gins = [nc.dram_tensor(f"gin{g}", [MG[g], 6 * len(GRP[g]) * 128], f16,
                           kind="ExternalInput") for g in range(len(GRP))]
    oins = [nc.dram_tensor(f"oin{g}", [MG[g], MG[g]], f16, kind="ExternalInput")
            for g in range(len(GRP))]
    gdrs = [nc.dram_tensor(f"gdr{g}", [REP, MG[g], BR, WO], f16, kind="Internal")
            for g in range(len(GRP))]
    out = nc.dram_tensor("out", [128, HOC, WO], f16, kind="ExternalOutput")

    with tile.TileContext(nc) as tc:
        from contextlib import ExitStack
        with ExitStack() as ctx:
            big = ctx.enter_context(tc.tile_pool(name="big", bufs=1))
            gsb = ctx.enter_context(tc.tile_pool(name="gsb", bufs=2))
            gc_p = ctx.enter_context(tc.tile_pool(name="gc", bufs=2))
            yt_p = ctx.enter_context(tc.tile_pool(name="yt", bufs=2))
            tr_p = ctx.enter_context(tc.tile_pool(name="tr", bufs=2))
            ps_a = ctx.enter_context(tc.tile_pool(name="ps_a", bufs=2, space="PSUM"))
            ps_g = ctx.enter_context(tc.tile_pool(name="ps_g", bufs=2, space="PSUM"))

            # --- load order matters. sync + scalar are the HW-DGE queues (fast);
            # the gpsimd queue is software-DGE (slow) -> tiny consts only.
            # scalar queue is free until the first ACT copy (~16us), so it takes
            # the group-A weights (tiny, unblocks warmup) then half the x chunks.
            ws = big.tile([128, 9 * 18], f16)
            with tc.high_priority():
                nc.scalar.dma_start(out=ws[:], in_=win[:])

            xsk = []
            for blk in range(NBLK):
                xs = big.tile([128, CR, 2, JW], f16, tag=f"xs{blk}")
                xsk.append(xs)

            def xdma(eng, blk):
                # whole chunk in one DMA: fully contiguous per partition
                # (4488B single descriptor) for max DMA efficiency
                r0 = 16 * blk
                eng.dma_start(out=xsk[blk][:], in_=xin[:, r0:r0 + CR, :, :])

            xdma(nc.sync, 0)
            xdma(nc.sync, 1)
            xdma(nc.scalar, 2)
            xdma(nc.scalar, 3)
            cstg, osg, gsg = [], [], []
            off = 0
            for g in range(len(GRP)):
                cst = big.tile([MG[g], 2], f32, tag=f"cst{g}")
                nc.gpsimd.dma_start(out=cst[:], in_=cin[off:off + MG[g]])
                cstg.append(cst)
                off += MG[g]
                osn = big.tile([MG[g], MG[g]], f16, tag=f"osn{g}")
                nc.gpsimd.dma_start(out=osn[:], in_=oins[g][:])
                osg.append(osn)
                gst = big.tile([MG[g], 6 * len(GRP[g]) * 128], f16, tag=f"gs{g}")
                nc.gpsimd.dma_start(out=gst[:], in_=gins[g][:])
                gsg.append(gst)
            gdrg = gdrs

            def xtap(tap, blk):
                i, b = tap // 3, tap % 3
                if b == 2:
                    # plane 2 content == plane 0 shifted one j-slot; read the
                    # shifted view so conv never waits on the plane-2 copy
                    return xsk[blk][:, i:i + 2 * BR - 1:2, 0, 1:WO + 1]
                return xsk[blk][:, i:i + 2 * BR - 1:2, b, 0:WO]       # [128, 8, 64]

            def xtaps2(i, blk):
                # [128, 2, 8, 64]: (plane, row, col) for row-offset i, planes 0-1
                return xsk[blk][:, i:i + 2 * BR - 1:2, 0:2, 0:WO].transpose([0, 2, 1, 3])

            # ---- PE warm-up on the early tiny weights (p-state ramp) ----
            wu = ps_a.tile([MX, NPOS], f32, tag="ps")
            for _ in range(NWU):
                nc.tensor.matmul(wu[0:18, 0:9 * 18], ws[:, 0:18],
                                 ws[:], start=True, stop=True)


            # ---- conv per group; scheduling floors order the engine queues
            # (in-order queues suffer head-of-line blocking otherwise) ----
            PH_CONV = [[0.001, 0.002], [0.003, 0.004]]
            sigs = []
            for gi, blocks in enumerate(GRP):
                sig = ps_a.tile([MX, NPOS], f32, tag="ps")
                if len(blocks) > 1:
                    # banded layout: define the pad rows between bands so the
                    # full-width g-chain reads finite data (conv matmuls with
                    # start=True then overwrite the real bands)
                    nc.vector.memset(sig[:], 1.0)
                for b2, blk in enumerate(blocks):
                    with tc.tile_wait_until(PH_CONV[gi][b2]):
                        for tap in range(9):
                            # per-b2 accumulation groups (9 matmuls each, banded
                            # at partition 32*b2) keep the PE stream interruptible
                            nc.tensor.matmul(
                                sig[32 * b2:32 * b2 + 18],
                                ws[:, tap * 18:tap * 18 + 18],
                                xtap(tap, blk),
                                start=(tap == 0), stop=(tap == 8),
                                tile_position=(0, 32 * b2),
                            )
                sigs.append(sig)

            def g_emit(gi):
                mg = MG[gi]
                cst = cstg[gi]
                sig = sigs[gi]
                sc = gsb.tile([MX, NPOS], f32, tag="sc")
                nc.vector.tensor_scalar(out=sc[0:mg], in0=sig[0:mg],
                                        scalar1=cst[:, 1:2],
                                        scalar2=float(SIGMA_MIN),
                                        op0=ADD, op1=MAX)
                inv = gsb.tile([MX, NPOS], f32, tag="inv")
                nc.vector.reciprocal_approx_fast(out=inv[0:mg], in_=sc[0:mg])
                qt = gsb.tile([MX, NPOS], f32, tag="qt")
                nc.scalar.activation(out=qt[0:mg], in_=inv[0:mg], func=AF.Square)
                et = gsb.tile([MX, NPOS], f32, tag="et")
                nc.scalar.activation(out=et[0:mg], in_=qt[0:mg], func=AF.Exp,
                                     scale=cst[:, 0:1])
                gb = gsb.tile([MX, NPOS], f16, tag="gb")
                nc.vector.tensor_tensor(out=gb[0:mg], in0=et[0:mg], in1=inv[0:mg],
                                        op=MULT)
                S = ps_a.tile([MX, NPOS], f32, tag="ps")
                nc.tensor.matmul(S[0:mg], osg[gi][:], gb[0:mg], start=True, stop=True)
                rs = gsb.tile([MX, NPOS], f32, tag="rs")
                nc.vector.reciprocal_approx_fast(out=rs[0:mg], in_=S[0:mg])
                gbn = gsb.tile([MX, NPOS], f16, tag="gbn")
                nc.vector.tensor_tensor(out=gbn[0:mg], in0=gb[0:mg], in1=rs[0:mg],
                                        op=MULT)
                # scratch replicas in DRAM for the tap-6..8 broadcast DMA
                for r in range(REP):
                    nc.gpsimd.dma_start(out=gdrg[gi][r], in_=gbn[0:mg])
                return gbn

            def unfold_emit(blk, gbn):
                gi = next(g for g, bl in enumerate(GRP) if blk in bl)
                nb = len(GRP[gi])
                b2 = blk - GRP[gi][0]
                gc = gc_p.tile([128, 9, BR, WO], f16, tag="gc")
                # taps 6-8: stride-0 broadcast DMA from the DRAM replicas
                gdr = gdrg[gi]
                for hh in range(HH):
                    r6 = 32 * b2 + 9 * hh + 6
                    nc.sync.dma_start(
                        out=gc[64 * hh:64 * hh + 32, 6:9],
                        in_=gdr[0, r6:r6 + 3].unsqueeze(0).broadcast_to([32, 3, BR, WO]))
                    nc.scalar.dma_start(
                        out=gc[64 * hh + 32:64 * hh + 64, 6:9],
                        in_=gdr[1, r6:r6 + 3].unsqueeze(0).broadcast_to([32, 3, BR, WO]))
                # taps 0-5: PE one-hot bcast through PSUM + ACT fp16 copy
                mg = MG[gi]
                for tri in range(2):
                    gp = ps_g.tile([128, 3, NPOS], f32, tag="gp")
                    for t in range(3):
                        tap = tri * 3 + t
                        nc.tensor.matmul(
                            gp[:, t],
                            gsg[gi][:, (tap * nb + b2) * 128:(tap * nb + b2 + 1) * 128],
                            gbn[0:mg], start=True, stop=True)
                    nc.scalar.activation(out=gc[:, 3 * tri:3 * tri + 3],
                                         in_=gp[:], func=AF.Copy)

                yt = yt_p.tile([128, 9, BR, WO], f16, tag="yt")
                for i in range(3):
                    nc.vector.tensor_tensor(out=yt[:, 3 * i:3 * i + 2],
                                            in0=xtaps2(i, blk),
                                            in1=gc[:, 3 * i:3 * i + 2], op=MULT)
                    nc.vector.tensor_tensor(out=yt[:, 3 * i + 2],
                                            in0=xtap(3 * i + 2, blk),
                                            in1=gc[:, 3 * i + 2], op=MULT)

                t4 = tr_p.tile([128, 4, BR, WO], f16, tag="t4")
                nc.vector.tensor_tensor(out=t4[:], in0=yt[:, 0:8:2], in1=yt[:, 1:8:2], op=ADD)
                t2 = tr_p.tile([128, 2, BR, WO], f16, tag="t2")
                nc.vector.tensor_tensor(out=t2[:], in0=t4[:, 0:4:2], in1=t4[:, 1:4:2], op=ADD)
                tA = tr_p.tile([128, BR, WO], f16, tag="tA")
                nc.vector.tensor_tensor(out=tA[:], in0=t2[:, 0], in1=t2[:, 1], op=ADD)
                y = tr_p.tile([128, BR, WO], f16, tag="y")
                nc.vector.tensor_tensor(out=y[:], in0=tA[:], in1=yt[:, 8], op=ADD)
                nc.sync.dma_start(out=out[:, BR * blk:BR * (blk + 1), :], in_=y[:])

            with tc.tile_wait_until(0.0045):
                gbnA = g_emit(0)
            with tc.tile_wait_until(0.005):
                unfold_emit(0, gbnA)
            with tc.tile_wait_until(0.0055):
                gbnB = g_emit(1)
            with tc.tile_wait_until(0.006):
                unfold_emit(1, gbnA)
            with tc.tile_wait_until(0.007):
                unfold_emit(2, gbnB)
            with tc.tile_wait_until(0.008):
                unfold_emit(3, gbnB)

    if not for_sim and not nc.is_finalized():
        nc.finalize()
    return nc


def _prep_inputs(x, conv_w, bn_gamma, bn_beta, bn_mean, bn_var):
    cst, wt, ohs, ones = _build_consts(conv_w, bn_gamma, bn_beta, bn_mean, bn_var)
    xp = np.pad(np.asarray(x, np.float32), ((0, 0), (0, 0), (1, 1), (1, 1)),
                mode="reflect").astype(np.float16)                    # [8,64,130,130]
    in_maps = []
    for n in range(N):
        xc = np.concatenate([xp[n, :, 0:RS, :], xp[n, :, 64:64 + RS, :]], axis=0)
        xpl = np.zeros((128, RS, 2, JW), np.float16)
        xpl[:, :, 0, 0:65] = xc[:, :, 0:130:2]
        xpl[:, :, 1, 0:65] = xc[:, :, 1:130:2]
        im = {"xin": xpl, "cin": cst, "win": wt}
        for g in range(len(GRP)):
            im[f"gin{g}"] = ohs[g]
            im[f"oin{g}"] = ones[g]
        in_maps.append(im)
    return in_maps


def _gather(results):
    out = np.empty((N, C, HO, WO), np.float32)
    for n in range(N):
        d = results[n]["out"].astype(np.float32)
        out[n, :, 0:HOC, :] = d[0:64]
        out[n, :, HOC:, :] = d[64:128]
    return out


def _enable_axon_trace():
    if _STATE.get("trace_hooked"):
        return
    import types
    import antenv
    from concourse import bass_utils
    mod = types.ModuleType("antenv.axon_hooks")
    mod._hook = None
    mod.set_axon_ntff_profile_hook = lambda h: setattr(mod, "_hook", h)
    mod.get_axon_ntff_profile_hook = lambda: mod._hook
    sys.modules["antenv.axon_hooks"] = mod
    antenv.axon_hooks = mod
    from trn_agent_boot.trn_boot import _ntff_profile_via_ctypes
    mod._hook = _ntff_profile_via_ctypes("/opt/axon/libaxon_pjrt.so")
    bass_utils.upload_artifacts = lambda tmpdir: tmpdir
    _STATE["trace_hooked"] = True


def run(x, conv_w, bn_gamma, bn_beta, bn_mean, bn_var, trace=False):
    from concourse.bass_utils import run_bass_kernel_spmd
    if trace:
        _enable_axon_trace()
    if "nc" not in _STATE:
        _STATE["nc"] = _build_bass()
    in_maps = _prep_inputs(x, conv_w, bn_gamma, bn_beta, bn_mean, bn_var)
    res = run_bass_kernel_spmd(_STATE["nc"], in_maps, list(range(N)), trace=trace)
    _STATE["last"] = res
    return _gather(res.results)


def kernel(x, conv_w, bn_gamma, bn_beta, bn_mean, bn_var):
    return run(x, conv_w, bn_gamma, bn_beta, bn_mean, bn_var,
               trace=bool(int(os.environ.get("KERNEL_TRACE", "0"))))
